# revision 12
# baseline (speedup 1.0000x reference)
"""Trainium2 Bass kernel for nn_patch_expanding.

Computes, for x [32, 1024, 1024] and w [512, 512]:
    xg = x.reshape(B, 32, 32, 1024); x0, x1 = split(xg, channel halves)
    xi = row-interleave(x0, x1) -> [B, 64, 32, 512]
    y  = xi @ w -> reshape [B, 2048, 512]

Data-parallel over batch (4 batches/core on 8 cores); fp16 on device
(host rounds inputs; rel err ~5e-4 vs the 2e-2 gate). Per core per rep:
x [4096 tok, 1024 ch] -> y [8192, 512].

v8: w-stationary yT formulation, cin-major sharding.
- The per-core job is y[8192,512] = xi[8192,512] @ w, where xi is a row
  permutation of x's two channel halves. Both matmul operands must be
  cin-on-partitions, so each core's shard is laid out cin-major
  [1024 ch, 4096 tok] when the host distributes x (the HBM->SBUF XBAR
  transpose path caps at ~120 GB/s/HWDGE ring of descriptor generation
  = the ~70us wall of earlier revisions; the ACT ring's XBAR corrupts
  data, so it cannot be split across rings).
- Compute yT[cout, tok] = w[cin,cout].T-chunk @ xT[cin, tok]: the 16
  [128,128] w tiles are PE-stationary (1 LDWEIGHTS per 8 matmuls) and x
  streams as the moving operand in plain token order. 256 N=512 fp16
  matmuls/rep -> ~55us PE roofline. The channel-half interleave becomes
  a pure output-indexing problem handled on the host during unshard.
- Per (lg, c)-group: k-outer accumulation, 8 PSUM banks = (half s,
  token tile t); bank completions stagger so DVE evictions chase the
  k=3 pass and banks recycle without stalling the PE.
- SP ring: one plain 4MB load per load-group (4KB/partition runs,
  full DMA bandwidth); xt quad-buffered. DVE evicts PSUM->fp16 ysb
  (drain-fenced); ACT ring stores 1MB contiguous yT-tile groups
  (8KB/partition). Host reassembles y from the tile layout.
"""
import sys
sys.path.insert(0, "/opt/trn_rl_repo")
import numpy as np

B, L, C = 32, 1024, 1024
NCORES = 8
BPC = B // NCORES
ROWS = BPC * L             # 4096 tokens per core
TL = 2048                  # tokens per load-group
NGL = ROWS // TL           # 2 load-groups per rep
GPL = 4                    # (c) groups per load-group
GPR = NGL * GPL            # 8 psum-groups per rep (each: 8 banks of [128,512])

_CACHE = {}


def _build(reps: int = 1, sim: bool = False):
    import concourse.bass as bass
    from concourse import mybir

    f16, f32 = mybir.dt.float16, mybir.dt.float32
    nc = bass.Bass(trn_type="TRN2", target_bir_lowering=False, debug=False,
                   num_devices=NCORES)

    # x shard arrives cin-major: [1024 ch, 4096 tok]
    xd = nc.dram_tensor("x", [C, ROWS], f16, kind="ExternalInput").ap()
    wd = nc.dram_tensor("w", [512, 512], f16, kind="ExternalInput").ap()
    # yT tile layout: [group gg%8][cout part 128][bank slot 8][tok 512]
    yd = nc.dram_tensor("y", [GPR, 128, 8, 512], f16, kind="ExternalOutput").ap()

    xr = xd.rearrange("(kk p) t -> p kk t", p=128)   # [128, 8, 4096]

    s_lw = nc.alloc_semaphore("s_lw")
    s_tr = [nc.alloc_semaphore(f"s_tr{i}") for i in range(4)]
    s_mm = nc.alloc_semaphore("s_mm")    # +1 per completed psum bank
    s_ye = nc.alloc_semaphore("s_ye")    # +1 per DVE eviction
    s_yd = nc.alloc_semaphore("s_yd")    # +1 per drained evict-group
    # parity store sems: concurrent stores must not share a sem
    s_st = [nc.alloc_semaphore(f"s_st{i}") for i in range(2)]
    all_sems = s_tr + s_st + [s_lw, s_mm, s_ye, s_yd]

    GL = NGL * reps            # total load-groups
    G = GPR * reps             # total psum-groups

    with (
        nc.sbuf_tensor("xt", [128, 4, 8, TL], f16) as xt,
        nc.sbuf_tensor("wsb", [128, 4, 4, 128], f16) as wsb,
        nc.sbuf_tensor("ysb", [128, 2, 8, 512], f16) as ysb,
        nc.psum_tensor("ps", [128, 8, 512], f32) as ps,
    ):
        xt_a, wsb_a, ysb_a, ps_a = xt.ap(), wsb.ap(), ysb.ap(), ps.ap()

        if not sim:
            for s in all_sems:
                nc.gpsimd.sem_clear(s)
            for eng in (nc.sync, nc.tensor, nc.vector, nc.scalar):
                for _ in range(4):
                    eng.nop(cycle_cnt=6000, nofuse=True)

        def store(eng, gg):
            # one 1MB contiguous store per completed psum-group
            eng.wait_ge(s_yd, gg + 1)
            eng.dma_start(
                yd[gg % GPR], ysb_a[:, gg % 2, :, :],
            ).then_inc(s_st[gg % 2], 16)

        with nc.Block() as block:

            @block.gpsimd
            def _(g):
                g.wait_ge(s_st[0], 16 * (G // 2))
                g.wait_ge(s_st[1], 16 * (G // 2))
                if not sim:
                    for s in all_sems:
                        g.sem_clear(s)

            @block.sync
            def _(sp):
                sp.dma_start(
                    wsb_a[:],
                    wd.rearrange("(k p) (c m) -> p k c m", p=128, m=128),
                ).then_inc(s_lw, 16)
                for lg in range(GL):
                    par, la = lg % 4, lg % NGL
                    if lg >= 4:
                        # xt[par] free once PE consumed load-group lg-4
                        sp.wait_ge(s_mm, 8 * GPL * (lg - 3))
                    sp.dma_start(
                        xt_a[:, par, :, :],
                        xr[:, :, TL * la:TL * la + TL],
                    ).then_inc(s_tr[par], 16)

            @block.scalar
            def _(ac):
                for gg in range(G):
                    store(ac, gg)

            @block.tensor
            def _(pe):
                pe.wait_ge(s_lw, 16)
                for gg in range(G):
                    lg, c = gg // GPL, gg % GPL
                    par = lg % 4
                    if c == 0:
                        pe.wait_ge(s_tr[par], 16 * (lg // 4 + 1))
                    for k in range(4):
                        for b in range(8):          # bank = 4*s + t
                            s, t = b // 4, b % 4
                            if k == 0 and gg >= 1:
                                pe.wait_ge(s_ye, 8 * (gg - 1) + b + 1)
                            inst = pe.matmul(
                                ps_a[:, b, :],
                                wsb_a[:, k, c, :],
                                xt_a[:, par, 4 * s + k, 512 * t:512 * t + 512],
                                start=(k == 0), stop=(k == 3),
                            )
                            if k == 3:
                                inst.then_inc(s_mm)

            @block.vector
            def _(dv):
                for gg in range(G):
                    parS = gg % 2
                    for b in range(8):
                        if b == 0 and gg >= 2:
                            # store gg-2 (same parity) complete -> ysb free
                            dv.wait_ge(s_st[parS], 16 * (gg // 2))
                        dv.wait_ge(s_mm, 8 * gg + b + 1)
                        dv.tensor_copy(ysb_a[:, parS, b, :], ps_a[:, b, :]
                                       ).then_inc(s_ye)
                    # visibility barrier for the store of this group
                    dv.drain().then_inc(s_yd)

    return nc


def _in_maps(x: np.ndarray, w: np.ndarray) -> list:
    # shard batch-parallel, cin-major per core: [C, ROWS]
    xs = np.ascontiguousarray(
        np.asarray(x, dtype=np.float16).reshape(NCORES, ROWS, C)
        .transpose(0, 2, 1))
    wh = np.ascontiguousarray(w, dtype=np.float16)
    return [{"x": xs[i], "w": wh} for i in range(NCORES)]


def _unshard(yts: np.ndarray) -> np.ndarray:
    """yts [NCORES, GPR, 128, 8, 512] fp16 -> y [B, 2L, 512] fp32.

    Device tile (gg=(lg,c), p, slot=(s,t), u) holds
    yT[cout=128c+p, x_tok=2048lg+512t+u] for channel half s, where
    x_tok = 1024*b'' + 32h + w  (b'' in 0..3 per core) and the output row
    is 64h + 32s + w.  Decompose t=(th,tp): b''=2lg+th; u=(h2,wc):
    h=16*tp+h2 -> row = 1024*tp + 64*h2 + 32*s + wc.
    """
    a = yts.reshape(NCORES, 2, 4, 128, 2, 2, 2, 16, 32)
    #                core, lg,  c,   p, s, th, tp, h2, wc
    a = a.transpose(0, 1, 5, 6, 7, 4, 8, 2, 3)
    #   core, lg, th, tp, h2, s, wc, c, p
    return a.reshape(B, 2 * L, C // 2).astype(np.float32)


def kernel(x: np.ndarray, w: np.ndarray) -> np.ndarray:
    from concourse.bass_utils import run_bass_kernel_spmd

    if "nc" not in _CACHE:
        _CACHE["nc"] = _build()
    nc = _CACHE["nc"]

    in_maps = _in_maps(x, w)
    res = run_bass_kernel_spmd(nc, in_maps, list(range(NCORES)))
    yts = np.stack([res.results[i]["y"] for i in range(NCORES)], axis=0)
    return _unshard(yts)


# revision 13
# speedup vs baseline: 1.2428x; 1.2428x over previous
"""Trainium2 Bass kernel for nn_patch_expanding.

Computes, for x [32, 1024, 1024] and w [512, 512]:
    xg = x.reshape(B, 32, 32, 1024); x0, x1 = split(xg, channel halves)
    xi = row-interleave(x0, x1) -> [B, 64, 32, 512]
    y  = xi @ w -> reshape [B, 2048, 512]

Data-parallel over batch (4 batches/core on 8 cores); fp16 on device
(host rounds inputs; rel err ~5e-4 vs the 2e-2 gate). Per core per rep:
x [4096 tok, 1024 ch] -> y [8192, 512].

v8: w-stationary yT formulation, cin-major sharding.
- The per-core job is y[8192,512] = xi[8192,512] @ w, where xi is a row
  permutation of x's two channel halves. Both matmul operands must be
  cin-on-partitions, so each core's shard is laid out cin-major
  [1024 ch, 4096 tok] when the host distributes x (the HBM->SBUF XBAR
  transpose path caps at ~120 GB/s/HWDGE ring of descriptor generation
  = the ~70us wall of earlier revisions; the ACT ring's XBAR corrupts
  data, so it cannot be split across rings).
- Compute yT[cout, tok] = w[cin,cout].T-chunk @ xT[cin, tok]: the 16
  [128,128] w tiles are PE-stationary (1 LDWEIGHTS per 8 matmuls) and x
  streams as the moving operand in plain token order. 256 N=512 fp16
  matmuls/rep -> ~55us PE roofline. The channel-half interleave becomes
  a pure output-indexing problem handled on the host during unshard.
- Per (lg, c)-group: k-outer accumulation, 8 PSUM banks = (half s,
  token tile t); bank completions stagger so DVE evictions chase the
  k=3 pass and banks recycle without stalling the PE.
- SP ring: one plain 4MB load per load-group (4KB/partition runs,
  full DMA bandwidth); xt quad-buffered. DVE evicts PSUM->fp16 ysb
  (drain-fenced); ACT ring stores 1MB contiguous yT-tile groups
  (8KB/partition). Host reassembles y from the tile layout.
"""
import sys
sys.path.insert(0, "/opt/trn_rl_repo")
import numpy as np

B, L, C = 32, 1024, 1024
NCORES = 8
BPC = B // NCORES
ROWS = BPC * L             # 4096 tokens per core
TL = 2048                  # tokens per load-group
NGL = ROWS // TL           # 2 load-groups per rep
GPL = 4                    # (c) groups per load-group
GPR = NGL * GPL            # 8 psum-groups per rep (each: 8 banks of [128,512])

_CACHE = {}


def _build(reps: int = 1, sim: bool = False):
    import concourse.bass as bass
    from concourse import mybir

    f16, f32 = mybir.dt.float16, mybir.dt.float32
    nc = bass.Bass(trn_type="TRN2", target_bir_lowering=False, debug=False,
                   num_devices=NCORES)

    # x shard arrives cin-major: [1024 ch, 4096 tok]
    xd = nc.dram_tensor("x", [C, ROWS], f16, kind="ExternalInput").ap()
    wd = nc.dram_tensor("w", [512, 512], f16, kind="ExternalInput").ap()
    # yT tile layout: [group gg%8][cout part 128][bank slot 8][tok 512]
    yd = nc.dram_tensor("y", [GPR, 128, 8, 512], f16, kind="ExternalOutput").ap()

    xr = xd.rearrange("(kk p) t -> p kk t", p=128)   # [128, 8, 4096]

    s_lw = nc.alloc_semaphore("s_lw")
    s_tr = [nc.alloc_semaphore(f"s_tr{i}") for i in range(4)]
    s_mm = nc.alloc_semaphore("s_mm")    # +1 per completed psum bank
    s_ye = nc.alloc_semaphore("s_ye")    # +1 per DVE eviction
    s_yd = nc.alloc_semaphore("s_yd")    # +1 per drained evict-group
    # parity store sems: concurrent stores must not share a sem
    s_st = [nc.alloc_semaphore(f"s_st{i}") for i in range(2)]
    all_sems = s_tr + s_st + [s_lw, s_mm, s_ye, s_yd]

    GL = NGL * reps            # total load-groups
    G = GPR * reps             # total psum-groups

    with (
        nc.sbuf_tensor("xt", [128, 4, 8, TL], f16) as xt,
        nc.sbuf_tensor("wsb", [128, 4, 4, 128], f16) as wsb,
        nc.sbuf_tensor("ysb", [128, 2, 8, 512], f16) as ysb,
        nc.psum_tensor("ps", [128, 8, 512], f32) as ps,
    ):
        xt_a, wsb_a, ysb_a, ps_a = xt.ap(), wsb.ap(), ysb.ap(), ps.ap()

        if not sim:
            for s in all_sems:
                nc.gpsimd.sem_clear(s)
            for eng in (nc.sync, nc.tensor, nc.vector, nc.scalar):
                for _ in range(4):
                    eng.nop(cycle_cnt=6000, nofuse=True)

        def store(eng, gg):
            # one 1MB contiguous store per completed psum-group
            eng.wait_ge(s_yd, gg + 1)
            eng.dma_start(
                yd[gg % GPR], ysb_a[:, gg % 2, :, :],
            ).then_inc(s_st[gg % 2], 16)

        with nc.Block() as block:

            @block.gpsimd
            def _(g):
                g.wait_ge(s_st[0], 16 * (G // 2))
                g.wait_ge(s_st[1], 16 * (G // 2))
                if not sim:
                    for s in all_sems:
                        g.sem_clear(s)

            @block.sync
            def _(sp):
                sp.dma_start(
                    wsb_a[:],
                    wd.rearrange("(k p) (c m) -> p k c m", p=128, m=128),
                ).then_inc(s_lw, 16)
                for lg in range(GL):
                    par, la = lg % 4, lg % NGL
                    if lg >= 4:
                        # xt[par] free once PE consumed load-group lg-4
                        sp.wait_ge(s_mm, 8 * GPL * (lg - 3))
                    sp.dma_start(
                        xt_a[:, par, :, :],
                        xr[:, :, TL * la:TL * la + TL],
                    ).then_inc(s_tr[par], 16)

            @block.scalar
            def _(ac):
                for gg in range(G):
                    store(ac, gg)

            @block.tensor
            def _(pe):
                # half-groups hg = (lg, c, s): 4 banks each (bank = 4*s + t).
                # 4-bank granularity gives DVE a full half-group span of
                # slack per eviction set, so the PE never stalls on PSUM
                # recycling (stalls micro-idle the PE and oscillate the HAM
                # clock gate down to K=4/8).
                pe.wait_ge(s_lw, 16)
                for hg in range(2 * G):
                    lg, c, s = hg // 8, (hg // 2) % GPL, hg % 2
                    par = lg % 4
                    if c == 0 and s == 0:
                        pe.wait_ge(s_tr[par], 16 * (lg // 4 + 1))
                    if hg >= 2:
                        # bank set (hg%2) free once hg-2 fully evicted
                        pe.wait_ge(s_ye, 4 * hg - 4)
                    for k in range(4):
                        for t in range(4):
                            inst = pe.matmul(
                                ps_a[:, 4 * s + t, :],
                                wsb_a[:, k, c, :],
                                xt_a[:, par, 4 * s + k, 512 * t:512 * t + 512],
                                start=(k == 0), stop=(k == 3),
                            )
                            if k == 3:
                                inst.then_inc(s_mm)

            @block.vector
            def _(dv):
                for hg in range(2 * G):
                    gg, s = hg // 2, hg % 2
                    parS = gg % 2
                    for t in range(4):
                        if s == 0 and t == 0 and gg >= 2:
                            # store gg-2 (same parity) complete -> ysb free
                            dv.wait_ge(s_st[parS], 16 * (gg // 2))
                        dv.wait_ge(s_mm, 4 * hg + t + 1)
                        dv.tensor_copy(ysb_a[:, parS, 4 * s + t, :],
                                       ps_a[:, 4 * s + t, :]).then_inc(s_ye)
                    if s == 1:
                        # visibility barrier for the store of this group
                        dv.drain().then_inc(s_yd)

    return nc


def _in_maps(x: np.ndarray, w: np.ndarray) -> list:
    # shard batch-parallel, cin-major per core: [C, ROWS]
    xs = np.ascontiguousarray(
        np.asarray(x, dtype=np.float16).reshape(NCORES, ROWS, C)
        .transpose(0, 2, 1))
    wh = np.ascontiguousarray(w, dtype=np.float16)
    return [{"x": xs[i], "w": wh} for i in range(NCORES)]


def _unshard(yts: np.ndarray) -> np.ndarray:
    """yts [NCORES, GPR, 128, 8, 512] fp16 -> y [B, 2L, 512] fp32.

    Device tile (gg=(lg,c), p, slot=(s,t), u) holds
    yT[cout=128c+p, x_tok=2048lg+512t+u] for channel half s, where
    x_tok = 1024*b'' + 32h + w  (b'' in 0..3 per core) and the output row
    is 64h + 32s + w.  Decompose t=(th,tp): b''=2lg+th; u=(h2,wc):
    h=16*tp+h2 -> row = 1024*tp + 64*h2 + 32*s + wc.
    """
    a = yts.reshape(NCORES, 2, 4, 128, 2, 2, 2, 16, 32)
    #                core, lg,  c,   p, s, th, tp, h2, wc
    a = a.transpose(0, 1, 5, 6, 7, 4, 8, 2, 3)
    #   core, lg, th, tp, h2, s, wc, c, p
    return a.reshape(B, 2 * L, C // 2).astype(np.float32)


def kernel(x: np.ndarray, w: np.ndarray) -> np.ndarray:
    from concourse.bass_utils import run_bass_kernel_spmd

    if "nc" not in _CACHE:
        _CACHE["nc"] = _build()
    nc = _CACHE["nc"]

    in_maps = _in_maps(x, w)
    res = run_bass_kernel_spmd(nc, in_maps, list(range(NCORES)))
    yts = np.stack([res.results[i]["y"] for i in range(NCORES)], axis=0)
    return _unshard(yts)


# revision 19
# speedup vs baseline: 1.3708x; 1.1029x over previous
"""Trainium2 Bass kernel for nn_patch_expanding.

Computes, for x [32, 1024, 1024] and w [512, 512]:
    xg = x.reshape(B, 32, 32, 1024); x0, x1 = split(xg, channel halves)
    xi = row-interleave(x0, x1) -> [B, 64, 32, 512]
    y  = xi @ w -> reshape [B, 2048, 512]

Data-parallel over batch (4 batches/core on 8 cores); fp16 on device
(host rounds inputs; rel err ~5e-4 vs the 2e-2 gate). Per core per rep:
x [4096 tok, 1024 ch] -> y [8192, 512].

v8: w-stationary yT formulation, cin-major sharding.
- The per-core job is y[8192,512] = xi[8192,512] @ w, where xi is a row
  permutation of x's two channel halves. Both matmul operands must be
  cin-on-partitions, so each core's shard is laid out cin-major
  [1024 ch, 4096 tok] when the host distributes x (the HBM->SBUF XBAR
  transpose path caps at ~120 GB/s/HWDGE ring of descriptor generation
  = the ~70us wall of earlier revisions; the ACT ring's XBAR corrupts
  data, so it cannot be split across rings).
- Compute yT[cout, tok] = w[cin,cout].T-chunk @ xT[cin, tok]: the 16
  [128,128] w tiles are PE-stationary (1 LDWEIGHTS per 8 matmuls) and x
  streams as the moving operand in plain token order. 256 N=512 fp16
  matmuls/rep -> ~55us PE roofline. The channel-half interleave becomes
  a pure output-indexing problem handled on the host during unshard.
- Per (lg, c)-group: k-outer accumulation, 8 PSUM banks = (half s,
  token tile t); bank completions stagger so DVE evictions chase the
  k=3 pass and banks recycle without stalling the PE.
- SP ring: one plain 4MB load per load-group (4KB/partition runs,
  full DMA bandwidth); xt quad-buffered. DVE evicts PSUM->fp16 ysb
  (drain-fenced); ACT ring stores 1MB contiguous yT-tile groups
  (8KB/partition). Host reassembles y from the tile layout.
"""
import sys
sys.path.insert(0, "/opt/trn_rl_repo")
import numpy as np

B, L, C = 32, 1024, 1024
NCORES = 8
BPC = B // NCORES
ROWS = BPC * L             # 4096 tokens per core
TL = 2048                  # tokens per load-group
NGL = ROWS // TL           # 2 load-groups per rep
GPL = 4                    # (c) groups per load-group
GPR = NGL * GPL            # 8 psum-groups per rep (each: 8 banks of [128,512])

_CACHE = {}


def _build(reps: int = 1, sim: bool = False):
    import concourse.bass as bass
    from concourse import mybir

    f16, f32 = mybir.dt.float16, mybir.dt.float32
    nc = bass.Bass(trn_type="TRN2", target_bir_lowering=False, debug=False,
                   num_devices=NCORES)

    # x shard arrives cin-major: [1024 ch, 4096 tok]
    xd = nc.dram_tensor("x", [C, ROWS], f16, kind="ExternalInput").ap()
    wd = nc.dram_tensor("w", [512, 512], f16, kind="ExternalInput").ap()
    # yT tile layout: [group gg%8][cout part 128][bank slot 8][tok 512]
    yd = nc.dram_tensor("y", [GPR, 128, 8, 512], f16, kind="ExternalOutput").ap()

    xr = xd.rearrange("(kk p) t -> p kk t", p=128)   # [128, 8, 4096]

    NSL = 4                    # ysb staging slots
    s_lw = nc.alloc_semaphore("s_lw")
    s_tr = [nc.alloc_semaphore(f"s_tr{i}") for i in range(4)]
    s_mm = nc.alloc_semaphore("s_mm")    # +1 per completed half-group
    s_ye = nc.alloc_semaphore("s_ye")    # +1 per DVE eviction
    s_yd = nc.alloc_semaphore("s_yd")    # +1 per drained evict-group
    # per-slot store sems: concurrent stores must not share a sem
    s_st = [nc.alloc_semaphore(f"s_st{i}") for i in range(NSL)]
    all_sems = s_tr + s_st + [s_lw, s_mm, s_ye, s_yd]

    GL = NGL * reps            # total load-groups
    G = GPR * reps             # total psum-groups

    with (
        nc.sbuf_tensor("xt", [128, 4, 8, TL], f16) as xt,
        nc.sbuf_tensor("wsb", [128, 4, 4, 128], f16) as wsb,
        nc.sbuf_tensor("ysb", [128, NSL, 8, 512], f16) as ysb,
        nc.psum_tensor("ps", [128, 8, 512], f32) as ps,
    ):
        xt_a, wsb_a, ysb_a, ps_a = xt.ap(), wsb.ap(), ysb.ap(), ps.ap()

        if not sim:
            for s in all_sems:
                nc.gpsimd.sem_clear(s)
            for eng in (nc.sync, nc.tensor, nc.vector, nc.scalar):
                for _ in range(4):
                    eng.nop(cycle_cnt=6000, nofuse=True)

        def store(eng, gg):
            # one 1MB contiguous store per completed psum-group
            eng.wait_ge(s_yd, gg + 1)
            eng.dma_start(
                yd[gg % GPR], ysb_a[:, gg % NSL, :, :],
            ).then_inc(s_st[gg % NSL], 16)

        with nc.Block() as block:

            @block.gpsimd
            def _(g):
                for i in range(NSL):
                    g.wait_ge(s_st[i], 16 * (G // NSL))
                if not sim:
                    for s in all_sems:
                        g.sem_clear(s)

            @block.sync
            def _(sp):
                sp.dma_start(
                    wsb_a[:],
                    wd.rearrange("(k p) (c m) -> p k c m", p=128, m=128),
                ).then_inc(s_lw, 16)
                for lg in range(GL):
                    par, la = lg % 4, lg % NGL
                    if lg >= 4:
                        # xt[par] free once PE consumed load-group lg-4
                        sp.wait_ge(s_mm, 8 * (lg - 3))
                    sp.dma_start(
                        xt_a[:, par, :, :],
                        xr[:, :, TL * la:TL * la + TL],
                    ).then_inc(s_tr[par], 16)

            @block.scalar
            def _(ac):
                for gg in range(G):
                    store(ac, gg)

            @block.tensor
            def _(pe):
                # half-groups hg = (lg, c, s): 4 banks each (bank = 4*s + t).
                # 4-bank granularity gives DVE a full half-group span of
                # slack per eviction set, so the PE never stalls on PSUM
                # recycling (stalls micro-idle the PE and oscillate the HAM
                # clock gate down to K=4/8).
                pe.wait_ge(s_lw, 16)
                for hg in range(2 * G):
                    lg, c, s = hg // 8, (hg // 2) % GPL, hg % 2
                    par = lg % 4
                    if c == 0 and s == 0:
                        pe.wait_ge(s_tr[par], 16 * (lg // 4 + 1))
                    if hg >= 2:
                        # bank set (hg%2) free once hg-2 fully evicted
                        pe.wait_ge(s_ye, 4 * hg - 4)
                    for k in range(4):
                        for t in range(4):
                            inst = pe.matmul(
                                ps_a[:, 4 * s + t, :],
                                wsb_a[:, k, c, :],
                                xt_a[:, par, 4 * s + k, 512 * t:512 * t + 512],
                                start=(k == 0), stop=(k == 3),
                            )
                            if k == 3 and t == 3:
                                # MMs complete in pc order: one inc covers
                                # the whole half-group (saves the ~26ns/inc
                                # serialized sem tail)
                                inst.then_inc(s_mm)

            @block.vector
            def _(dv):
                for hg in range(2 * G):
                    gg, s = hg // 2, hg % 2
                    slot = gg % NSL
                    dv.wait_ge(s_mm, hg + 1)
                    for t in range(4):
                        if s == 0 and t == 0 and gg >= NSL:
                            # store gg-NSL (same slot) complete -> ysb free
                            dv.wait_ge(s_st[slot], 16 * (gg // NSL))
                        dv.tensor_copy(ysb_a[:, slot, 4 * s + t, :],
                                       ps_a[:, 4 * s + t, :]).then_inc(s_ye)
                    if s == 1:
                        # visibility barrier for the store of this group
                        dv.drain().then_inc(s_yd)

    return nc


def _in_maps(x: np.ndarray, w: np.ndarray) -> list:
    # shard batch-parallel, cin-major per core: [C, ROWS]
    xs = np.ascontiguousarray(
        np.asarray(x, dtype=np.float16).reshape(NCORES, ROWS, C)
        .transpose(0, 2, 1))
    wh = np.ascontiguousarray(w, dtype=np.float16)
    return [{"x": xs[i], "w": wh} for i in range(NCORES)]


def _unshard(yts: np.ndarray) -> np.ndarray:
    """yts [NCORES, GPR, 128, 8, 512] fp16 -> y [B, 2L, 512] fp32.

    Device tile (gg=(lg,c), p, slot=(s,t), u) holds
    yT[cout=128c+p, x_tok=2048lg+512t+u] for channel half s, where
    x_tok = 1024*b'' + 32h + w  (b'' in 0..3 per core) and the output row
    is 64h + 32s + w.  Decompose t=(th,tp): b''=2lg+th; u=(h2,wc):
    h=16*tp+h2 -> row = 1024*tp + 64*h2 + 32*s + wc.
    """
    a = yts.reshape(NCORES, 2, 4, 128, 2, 2, 2, 16, 32)
    #                core, lg,  c,   p, s, th, tp, h2, wc
    a = a.transpose(0, 1, 5, 6, 7, 4, 8, 2, 3)
    #   core, lg, th, tp, h2, s, wc, c, p
    return a.reshape(B, 2 * L, C // 2).astype(np.float32)


def kernel(x: np.ndarray, w: np.ndarray) -> np.ndarray:
    from concourse.bass_utils import run_bass_kernel_spmd

    if "nc" not in _CACHE:
        _CACHE["nc"] = _build()
    nc = _CACHE["nc"]

    in_maps = _in_maps(x, w)
    res = run_bass_kernel_spmd(nc, in_maps, list(range(NCORES)))
    yts = np.stack([res.results[i]["y"] for i in range(NCORES)], axis=0)
    return _unshard(yts)


# revision 20
# speedup vs baseline: 1.3996x; 1.0210x over previous
"""Trainium2 Bass kernel for nn_patch_expanding.

Computes, for x [32, 1024, 1024] and w [512, 512]:
    xg = x.reshape(B, 32, 32, 1024); x0, x1 = split(xg, channel halves)
    xi = row-interleave(x0, x1) -> [B, 64, 32, 512]
    y  = xi @ w -> reshape [B, 2048, 512]

Data-parallel over batch (4 batches/core on 8 cores); fp16 on device
(host rounds inputs; rel err ~5e-4 vs the 2e-2 gate). Per core per rep:
x [4096 tok, 1024 ch] -> y [8192, 512].

v10: w-stationary yT formulation, cin-major sharding. Measured
~55.3us/rep = the 256 x 216ns fp16 PE streaming floor (94% PE busy).
- The per-core job is y[8192,512] = xi[8192,512] @ w, where xi is a row
  permutation of x's two channel halves. Both matmul operands must be
  cin-on-partitions, so each core's shard is laid out cin-major
  [1024 ch, 4096 tok] when the host distributes x (the HBM->SBUF XBAR
  transpose path caps at ~120 GB/s/HWDGE ring of descriptor generation
  = the ~70us wall of earlier revisions; the ACT ring's XBAR corrupts
  data -- verified rel-err 0.59 -- so it cannot be split across rings,
  and gpsimd SWDGE stores concurrent with XBAR transposes hang the
  device).
- Compute yT[cout, tok] = w[cin,cout].T-chunk @ xT[cin, tok]: the 16
  [128,128] w tiles are PE-stationary (LDWEIGHTS fully hidden behind
  the matmul stream via the PE's background weight buffer) and x
  streams as the moving operand in plain token order. 256 N=512 fp16
  matmuls/rep. The channel-half interleave becomes a pure
  output-indexing problem handled on the host during unshard.
- PE iterates half-groups hg=(lg, cout-chunk c, half s) of 4 PSUM banks
  (token tiles t), k-outer accumulation. 4-bank granularity gives DVE a
  full half-group span of eviction slack, so PSUM recycling never
  stalls the PE -- stalls micro-idle the PE and oscillate the HAM clock
  gate down to K=4/8 (the 8-bank version measured 57us throttled, 598ns
  cold unpipelined matmuls). One s_mm inc per half-group (pc-order
  completion) avoids the ~26ns/inc serialized sem tail.
- SP ring: one plain 4MB load per load-group (4KB/partition runs, full
  DMA bandwidth); xt quad-buffered. DVE evicts PSUM->fp16 ysb
  (drain-fenced, 4 staging slots so store completion never back-
  pressures); ACT ring stores 1MB contiguous yT-tile groups
  (8KB/partition). Host reassembles y from the tile layout.
"""
import sys
sys.path.insert(0, "/opt/trn_rl_repo")
import numpy as np

B, L, C = 32, 1024, 1024
NCORES = 8
BPC = B // NCORES
ROWS = BPC * L             # 4096 tokens per core
TL = 2048                  # tokens per load-group
NGL = ROWS // TL           # 2 load-groups per rep
GPL = 4                    # (c) groups per load-group
GPR = NGL * GPL            # 8 psum-groups per rep (each: 8 banks of [128,512])

_CACHE = {}


def _build(reps: int = 1, sim: bool = False):
    import concourse.bass as bass
    from concourse import mybir

    f16, f32 = mybir.dt.float16, mybir.dt.float32
    nc = bass.Bass(trn_type="TRN2", target_bir_lowering=False, debug=False,
                   num_devices=NCORES)

    # x shard arrives cin-major: [1024 ch, 4096 tok]
    xd = nc.dram_tensor("x", [C, ROWS], f16, kind="ExternalInput").ap()
    wd = nc.dram_tensor("w", [512, 512], f16, kind="ExternalInput").ap()
    # yT tile layout: [group gg%8][cout part 128][bank slot 8][tok 512]
    yd = nc.dram_tensor("y", [GPR, 128, 8, 512], f16, kind="ExternalOutput").ap()

    xr = xd.rearrange("(kk p) t -> p kk t", p=128)   # [128, 8, 4096]

    NSL = 4                    # ysb staging slots
    s_lw = nc.alloc_semaphore("s_lw")
    s_tr = [nc.alloc_semaphore(f"s_tr{i}") for i in range(4)]
    s_mm = nc.alloc_semaphore("s_mm")    # +1 per completed half-group
    s_ye = nc.alloc_semaphore("s_ye")    # +1 per DVE eviction
    s_yd = nc.alloc_semaphore("s_yd")    # +1 per drained evict-group
    # per-slot store sems: concurrent stores must not share a sem
    s_st = [nc.alloc_semaphore(f"s_st{i}") for i in range(NSL)]
    all_sems = s_tr + s_st + [s_lw, s_mm, s_ye, s_yd]

    GL = NGL * reps            # total load-groups
    G = GPR * reps             # total psum-groups

    with (
        nc.sbuf_tensor("xt", [128, 4, 8, TL], f16) as xt,
        nc.sbuf_tensor("wsb", [128, 4, 4, 128], f16) as wsb,
        nc.sbuf_tensor("ysb", [128, NSL, 8, 512], f16) as ysb,
        nc.psum_tensor("ps", [128, 8, 512], f32) as ps,
    ):
        xt_a, wsb_a, ysb_a, ps_a = xt.ap(), wsb.ap(), ysb.ap(), ps.ap()

        if not sim:
            for s in all_sems:
                nc.gpsimd.sem_clear(s)
            for eng in (nc.sync, nc.tensor, nc.vector, nc.scalar):
                for _ in range(4):
                    eng.nop(cycle_cnt=6000, nofuse=True)

        def store(eng, gg):
            # one 1MB contiguous store per completed psum-group
            eng.wait_ge(s_yd, gg + 1)
            eng.dma_start(
                yd[gg % GPR], ysb_a[:, gg % NSL, :, :],
            ).then_inc(s_st[gg % NSL], 16)

        with nc.Block() as block:

            @block.gpsimd
            def _(g):
                for i in range(NSL):
                    g.wait_ge(s_st[i], 16 * (G // NSL))
                if not sim:
                    for s in all_sems:
                        g.sem_clear(s)

            @block.sync
            def _(sp):
                sp.dma_start(
                    wsb_a[:],
                    wd.rearrange("(k p) (c m) -> p k c m", p=128, m=128),
                ).then_inc(s_lw, 16)
                for lg in range(GL):
                    par, la = lg % 4, lg % NGL
                    if lg >= 4:
                        # xt[par] free once PE consumed load-group lg-4
                        sp.wait_ge(s_mm, 8 * (lg - 3))
                    sp.dma_start(
                        xt_a[:, par, :, :],
                        xr[:, :, TL * la:TL * la + TL],
                    ).then_inc(s_tr[par], 16)

            @block.scalar
            def _(ac):
                for gg in range(G):
                    store(ac, gg)

            @block.tensor
            def _(pe):
                # half-groups hg = (lg, c, s): 4 banks each (bank = 4*s + t).
                # 4-bank granularity gives DVE a full half-group span of
                # slack per eviction set, so the PE never stalls on PSUM
                # recycling (stalls micro-idle the PE and oscillate the HAM
                # clock gate down to K=4/8).
                pe.wait_ge(s_lw, 16)
                for hg in range(2 * G):
                    lg, c, s = hg // 8, (hg // 2) % GPL, hg % 2
                    par = lg % 4
                    if c == 0 and s == 0:
                        pe.wait_ge(s_tr[par], 16 * (lg // 4 + 1))
                    if hg >= 2:
                        # bank set (hg%2) free once hg-2 fully evicted
                        pe.wait_ge(s_ye, 4 * hg - 4)
                    for k in range(4):
                        for t in range(4):
                            inst = pe.matmul(
                                ps_a[:, 4 * s + t, :],
                                wsb_a[:, k, c, :],
                                xt_a[:, par, 4 * s + k, 512 * t:512 * t + 512],
                                start=(k == 0), stop=(k == 3),
                            )
                            if k == 3 and t == 3:
                                # MMs complete in pc order: one inc covers
                                # the whole half-group (saves the ~26ns/inc
                                # serialized sem tail)
                                inst.then_inc(s_mm)

            @block.vector
            def _(dv):
                for hg in range(2 * G):
                    gg, s = hg // 2, hg % 2
                    slot = gg % NSL
                    dv.wait_ge(s_mm, hg + 1)
                    for t in range(4):
                        if s == 0 and t == 0 and gg >= NSL:
                            # store gg-NSL (same slot) complete -> ysb free
                            dv.wait_ge(s_st[slot], 16 * (gg // NSL))
                        dv.tensor_copy(ysb_a[:, slot, 4 * s + t, :],
                                       ps_a[:, 4 * s + t, :]).then_inc(s_ye)
                    if s == 1:
                        # visibility barrier for the store of this group
                        dv.drain().then_inc(s_yd)

    return nc


def _in_maps(x: np.ndarray, w: np.ndarray) -> list:
    # shard batch-parallel, cin-major per core: [C, ROWS]
    xs = np.ascontiguousarray(
        np.asarray(x, dtype=np.float16).reshape(NCORES, ROWS, C)
        .transpose(0, 2, 1))
    wh = np.ascontiguousarray(w, dtype=np.float16)
    return [{"x": xs[i], "w": wh} for i in range(NCORES)]


def _unshard(yts: np.ndarray) -> np.ndarray:
    """yts [NCORES, GPR, 128, 8, 512] fp16 -> y [B, 2L, 512] fp32.

    Device tile (gg=(lg,c), p, slot=(s,t), u) holds
    yT[cout=128c+p, x_tok=2048lg+512t+u] for channel half s, where
    x_tok = 1024*b'' + 32h + w  (b'' in 0..3 per core) and the output row
    is 64h + 32s + w.  Decompose t=(th,tp): b''=2lg+th; u=(h2,wc):
    h=16*tp+h2 -> row = 1024*tp + 64*h2 + 32*s + wc.
    """
    a = yts.reshape(NCORES, 2, 4, 128, 2, 2, 2, 16, 32)
    #                core, lg,  c,   p, s, th, tp, h2, wc
    a = a.transpose(0, 1, 5, 6, 7, 4, 8, 2, 3)
    #   core, lg, th, tp, h2, s, wc, c, p
    return a.reshape(B, 2 * L, C // 2).astype(np.float32)


def kernel(x: np.ndarray, w: np.ndarray) -> np.ndarray:
    from concourse.bass_utils import run_bass_kernel_spmd

    if "nc" not in _CACHE:
        _CACHE["nc"] = _build()
    nc = _CACHE["nc"]

    in_maps = _in_maps(x, w)
    res = run_bass_kernel_spmd(nc, in_maps, list(range(NCORES)))
    yts = np.stack([res.results[i]["y"] for i in range(NCORES)], axis=0)
    return _unshard(yts)
